# revision 1
# baseline (speedup 1.0000x reference)
"""CodebookLoRASTELinear forward on 8 Trainium2 NeuronCores.

out = x @ (W_q + D)^T
  D   = (lora_B @ lora_A) * (alpha/rank)
  cb  = codebook / max|codebook|
  S   = exp(scale_log)                     (per [o, i//128] group)
  q   = cb[searchsorted(midpoints(cb), (W+D)/S)]
      == cb0 + sum_k d_k * ((W+D) > t_k*S)      (S > 0)
  W_q = q * S

Column-parallel sharding: W / scale / lora_B rows (out_features) are split
across the 8 cores; x and lora_A are replicated; per-core outputs are
concatenated on the host (no collectives).

x is pre-transposed and packed on the host into [i%128, g, m] chunk order
(bf16), so phase C needs no on-chip transposes: the big matmul streams
out[m(128), o(512)] tiles directly from xT chunks (lhsT) against the
folded W_eff^T [128 (i%128), g, 512 (o)] (rhs), both bf16 (fp32 PSUM
accumulation; rel-err budget ~1.6e-3 vs the 2e-2 gate).

Quantization (phase B) runs on-device in natural [o, i] layout where the
per-(o, group) scale is a per-partition scalar; compare inputs stay exact
fp32 (host exp for S, full-fp32 PE matmul for D) so bucket decisions match
the reference. The elementwise chain is spread across ACT / GpSimd / DVE
with fused scalar_tensor_tensor ops, and each finished W_eff^T group is
immediately consumed by the first output chunk's matmul chains so the PE
stays busy during phase B.
"""

import numpy as np
import sys

for _p in ("/opt/trn_rl_repo",):
    if _p not in sys.path:
        sys.path.insert(0, _p)

import ml_dtypes  # noqa: E402
import concourse.mybir as mybir  # noqa: E402
import concourse.tile as tile  # noqa: E402
from concourse import bacc  # noqa: E402
from concourse.bass_utils import run_bass_kernel_spmd  # noqa: E402
from concourse.masks import make_identity  # noqa: E402
from contextlib import ExitStack  # noqa: E402

N_CORES = 8
M = 8192  # 4 * 2048 tokens
I = 4096  # in_features
O = 4096  # out_features
GROUP = 128
NG = I // GROUP  # 32 groups along i
RANK = 64
ALPHA_OVER_RANK = 32.0 / 64.0
OS = O // N_CORES  # 512 out features per core
NOB = OS // 128  # 4 output row blocks per core
MSB = 512  # m columns per x chunk
NMSB = M // MSB  # 16 chunks
NMB = M // 128  # 64 output row blocks

F32 = mybir.dt.float32
F32R = mybir.dt.float32r
BF16 = mybir.dt.bfloat16
ALU = mybir.AluOpType

_cache = {}


def _build_program(cb0, tk, dk, reps=1, variant=""):
    """cb0: smallest normalized codebook entry; tk: 3 bucket thresholds;
    dk: 3 successive codebook differences. All host floats baked in.
    variant: timing-experiment knobs ("" for the real kernel)."""
    nc = bacc.Bacc("TRN2", target_bir_lowering=False, debug=False)

    xt_d = nc.dram_tensor("xt", [NMSB * 128, NG, MSB], BF16,
                          kind="ExternalInput").ap()
    w_d = nc.dram_tensor("w", [OS, NG, GROUP], F32, kind="ExternalInput").ap()
    scl_d = nc.dram_tensor("scl", [OS, NG], F32, kind="ExternalInput").ap()
    la_d = nc.dram_tensor("la", [RANK, I], F32, kind="ExternalInput").ap()
    lbt_d = nc.dram_tensor("lbt", [RANK, OS], F32, kind="ExternalInput").ap()
    out_d = nc.dram_tensor("out", [M, OS], F32, kind="ExternalOutput").ap()

    with tile.TileContext(nc) as tc, ExitStack() as ctx:
        singles = ctx.enter_context(tc.tile_pool(name="singles", bufs=1))

        ident = singles.tile([128, 128], F32)
        make_identity(nc, ident)
        identr = singles.tile([128, 128], F32R)
        nc.vector.tensor_copy(identr, ident)

        # per-partition scale scalars S = exp(scale_log), natural
        # [o%128, ob, g] layout ("scl" already holds exp values -- a
        # sub-ulp host exp keeps quantization decisions aligned with the
        # reference; the on-chip ACT Exp table is ~2e-6 off, which flips
        # buckets near thresholds)
        ssc = singles.tile([128, NOB, NG], F32)  # S
        for ob in range(NOB):
            nc.sync.dma_start(
                out=ssc[:, ob, :], in_=scl_d[ob * 128 : (ob + 1) * 128, :]
            )
        d3_is_one = abs(float(dk[2]) - 1.0) < 1e-12
        # With d3 == 1 the middle indicator runs on ACT as
        # s2 = Sign(v - t2') in {-1,0,1}, contributing (d2/2)*s2 + d2/2;
        # the d2/2 constant folds into CC below. Otherwise CC = cb0.
        CC = float(cb0) + (float(dk[1]) / 2.0 if d3_is_one else 0.0)
        c0sc = singles.tile([128, NOB, NG], F32)  # CC * S
        nc.vector.tensor_scalar_mul(c0sc, ssc, float(CC))
        # shifted thresholds (t_k + CC) * S: phase B compares
        # v = W + D + CC*S against these, avoiding any subtract ops
        tsc = []
        for k in range(3):
            t = singles.tile([128, NOB, NG], F32, tag=f"tsc{k}")
            nc.vector.tensor_scalar_mul(t, ssc, float(tk[k] + CC))
            tsc.append(t)
        ntsc2 = singles.tile([128, NOB, NG], F32)  # -(t_1 + CC) * S, Sign bias
        nc.vector.tensor_scalar_mul(ntsc2, ssc, -float(tk[1] + CC))

        la_sb = singles.tile([RANK, I], F32)
        nc.sync.dma_start(out=la_sb, in_=la_d)
        lbt_sb = singles.tile([RANK, OS], F32)
        nc.sync.dma_start(out=lbt_sb, in_=lbt_d)
        # fold alpha/rank into B^T once
        nc.vector.tensor_scalar_mul(lbt_sb, lbt_sb, float(ALPHA_OVER_RANK))

        # persistent effective transposed weight, one tile per group so
        # phase C matmuls can consume groups as phase B finishes them
        weff = [
            singles.tile([128, OS], BF16, tag=f"weff{g}", name=f"weff{g}")
            for g in range(NG)
        ]

        if reps > 1 and variant != "hoistB":
            ctx.enter_context(tc.For_i(0, reps, 1))

        xpool = ctx.enter_context(tc.tile_pool(name="xpool", bufs=2))
        wload = ctx.enter_context(tc.tile_pool(name="wload", bufs=2))
        dpool = ctx.enter_context(tc.tile_pool(name="dpool", bufs=8))
        upool = ctx.enter_context(tc.tile_pool(name="upool", bufs=8))
        a1p = ctx.enter_context(tc.tile_pool(name="a1p", bufs=8))
        a2p = ctx.enter_context(tc.tile_pool(name="a2p", bufs=8))
        a12p = ctx.enter_context(tc.tile_pool(name="a12p", bufs=8))
        a123p = ctx.enter_context(tc.tile_pool(name="a123p", bufs=8))
        wqp = ctx.enter_context(tc.tile_pool(name="wqp", bufs=16))
        opool = ctx.enter_context(tc.tile_pool(name="opool", bufs=4))
        psumD = ctx.enter_context(tc.tile_pool(name="psumD", bufs=2, space="PSUM"))
        psumW = ctx.enter_context(tc.tile_pool(name="psumW", bufs=2, space="PSUM"))
        psumO = ctx.enter_context(tc.tile_pool(name="psumO", bufs=1, space="PSUM"))

        def load_chunk(msb):
            t = xpool.tile([128, NG, MSB], BF16, tag="xt")
            nc.sync.dma_start(out=t, in_=xt_d[msb * 128 : (msb + 1) * 128, :, :])
            return t

        NGQ = NG // 4  # W arrives in quads of groups

        def load_wquad(gq):
            ts = []
            for ob in range(NOB):
                t = wload.tile([128, 4, GROUP], F32, tag=f"w{ob}")
                nc.sync.dma_start(
                    out=t,
                    in_=w_d[ob * 128 : (ob + 1) * 128, gq * 4 : (gq + 1) * 4, :],
                )
                ts.append(t)
            return ts

        wq_tiles = {0: load_wquad(0), 1: load_wquad(1)}
        xt0 = load_chunk(0)
        xt1 = load_chunk(1)

        # output accumulation chains for chunk 0, fed group-by-group as
        # phase B completes each W_eff^T group
        chains0 = [
            psumO.tile([128, OS], F32, tag=f"o{mb}", name=f"chain{mb}")
            for mb in range(NOB)
        ]

        # ---- phase B: lora + quantize -> transpose -> weff[g], with the
        # first chunk's matmuls interleaved per finished group. The PE
        # stream is software-pipelined: lora matmuls for group g go out
        # with chunk-0 matmuls for g-2 and transposes for g-1, so the
        # ACT/GpSimd/DVE elementwise chain between a group's lora matmul
        # and its transpose has a full PE round to complete (otherwise
        # the in-order PE queue stalls on every tile).
        wq_sb = {}  # g -> quantized+lora'd natural-layout tiles, per ob

        def emit_lora(g):
            d_ps = psumD.tile([128, NOB, 128], F32, tag="d", name=f"dall{g}")
            for ob in range(NOB):
                nc.tensor.matmul(
                    d_ps[:, ob, :],
                    lhsT=lbt_sb[:, ob * 128 : (ob + 1) * 128],
                    rhs=la_sb[:, g * 128 : (g + 1) * 128],
                    start=True,
                    stop=True,
                )
            return d_ps

        def emit_elemwise(g, d_ps):
            gq, gi = divmod(g, 4)
            tiles = []
            for ob in range(NOB):
                sS = ssc[:, ob, g : g + 1]
                sC0 = c0sc[:, ob, g : g + 1]
                wn = wq_tiles[gq][ob][:, gi, :]
                # d_sb = D + CC*S (bias folded in during the PSUM read)
                d_sb = dpool.tile([128, 128], F32, tag="dsb")
                nc.scalar.activation(
                    d_sb, d_ps[:, ob, :], mybir.ActivationFunctionType.Identity,
                    bias=sC0, scale=1.0,
                )
                # v = W + D + CC*S, compared against (t_k + CC)*S below
                u = upool.tile([128, 128], F32, tag="u")
                nc.gpsimd.tensor_add(u, wn, d_sb)
                a1 = a1p.tile([128, 128], F32, tag="a1")
                nc.vector.tensor_scalar(
                    a1, u, tsc[0][:, ob, g : g + 1], float(dk[0]),
                    op0=ALU.is_gt, op1=ALU.mult,
                )
                a123 = a123p.tile([128, 128], F32, tag="a123")
                if d3_is_one:
                    # s2 = Sign(v - t2') on ACT; a12 = (d2/2)*s2 + a1;
                    # a123 = (v > t3') + a12
                    s2 = a2p.tile([128, 128], F32, tag="a2")
                    nc.scalar.activation(
                        s2, u, mybir.ActivationFunctionType.Sign,
                        bias=ntsc2[:, ob, g : g + 1], scale=1.0,
                    )
                    a12 = a12p.tile([128, 128], F32, tag="a12")
                    nc.vector.scalar_tensor_tensor(
                        a12, s2, float(dk[1]) / 2.0, a1,
                        op0=ALU.mult, op1=ALU.add,
                    )
                    nc.vector.scalar_tensor_tensor(
                        a123, u, tsc[2][:, ob, g : g + 1], a12,
                        op0=ALU.is_gt, op1=ALU.add,
                    )
                else:
                    a2 = a2p.tile([128, 128], F32, tag="a2")
                    nc.vector.tensor_scalar(
                        a2, u, tsc[1][:, ob, g : g + 1], float(dk[1]),
                        op0=ALU.is_gt, op1=ALU.mult,
                    )
                    a12 = a12p.tile([128, 128], F32, tag="a12")
                    nc.gpsimd.tensor_add(a12, a1, a2)
                    a3 = a123p.tile([128, 128], F32, tag="a3")
                    nc.vector.tensor_scalar(
                        a3, u, tsc[2][:, ob, g : g + 1], float(dk[2]),
                        op0=ALU.is_gt, op1=ALU.mult,
                    )
                    nc.gpsimd.tensor_add(a123, a12, a3)
                # w_eff = (q - CC)*S + (D + CC*S) = q*S + D
                wq_t = wqp.tile([128, 128], F32R, tag="wq")
                nc.vector.scalar_tensor_tensor(
                    wq_t, a123, sS, d_sb, op0=ALU.mult, op1=ALU.add
                )
                tiles.append(wq_t)
            wq_sb[g] = tiles

        def emit_transpose(g):
            wg = weff[g]
            for ob in range(NOB):
                pt = psumW.tile([128, 128], F32R, tag="pt")
                nc.tensor.transpose(pt, wq_sb[g][ob], identr)
                # cast-copy lands this ob's W_eff^T column block as bf16
                nc.scalar.copy(wg[:, ob * 128 : (ob + 1) * 128], pt.bitcast(F32))
            del wq_sb[g]

        def emit_mm0(g):
            # chunk 0 consumes finished groups two PE rounds behind
            for mb in range(NOB):
                nc.tensor.matmul(
                    chains0[mb],
                    lhsT=xt0[:, g, mb * 128 : (mb + 1) * 128],
                    rhs=weff[g],
                    start=(g == 0),
                    stop=(g == NG - 1),
                    skip_group_check=True,
                )

        T_LAG, M_LAG = 2, 5
        for g in range(NG + M_LAG):
            if g < NG:
                gq, gi = divmod(g, 4)
                if gi == 0 and gq + 2 < NGQ:
                    wq_tiles[gq + 2] = load_wquad(gq + 2)
                d_ps = emit_lora(g)
                emit_elemwise(g, d_ps)
            if M_LAG <= g < NG + M_LAG:
                emit_mm0(g - M_LAG)
            if T_LAG <= g < NG + T_LAG:
                emit_transpose(g - T_LAG)

        for mb in range(NOB):
            o_sb = opool.tile([128, OS], F32, tag="o")
            nc.scalar.copy(o_sb, chains0[mb])
            nc.sync.dma_start(out=out_d[mb * 128 : (mb + 1) * 128, :], in_=o_sb)

        # ---- phase C: stream remaining x chunks through the big matmul ----
        if reps > 1 and variant == "hoistB":
            ctx.enter_context(tc.For_i(0, reps, 1))
            xts = {1: load_chunk(1)}
        else:
            xts = {0: xt0, 1: xt1}
        for msb in range(1, NMSB):
            if msb + 1 < NMSB:
                xts[msb + 1] = load_chunk(msb + 1)
            xt_t = xts.pop(msb)
            for mb in range(NOB):
                mbg = msb * NOB + mb
                p_out = psumO.tile([128, OS], F32, tag=f"o{mb}")
                for g in range(NG):
                    nc.tensor.matmul(
                        p_out,
                        lhsT=xt_t[:, g, mb * 128 : (mb + 1) * 128],
                        rhs=weff[g],
                        start=(g == 0),
                        stop=(g == NG - 1),
                    )
                o_sb = opool.tile([128, OS], F32, tag="o")
                nc.scalar.copy(o_sb, p_out)
                nc.sync.dma_start(
                    out=out_d[mbg * 128 : (mbg + 1) * 128, :], in_=o_sb
                )

    nc.compile()
    return nc


def _get_program(cb0, tk, dk, reps=1, variant=""):
    key = (round(float(cb0), 9), tuple(round(float(t), 9) for t in tk),
           tuple(round(float(d), 9) for d in dk), reps, variant)
    if key not in _cache:
        _cache[key] = _build_program(cb0, tk, dk, reps, variant)
    return _cache[key]


def _codebook_consts(codebook):
    cb = np.asarray(codebook, dtype=np.float64)
    cb = cb / max(float(np.max(np.abs(cb))), 1e-8)
    tk = (cb[:-1] + cb[1:]) * 0.5
    dk = np.diff(cb)
    return float(cb[0]), [float(v) for v in tk], [float(v) for v in dk]


def _prep_in_maps(x, weight, scale_log, lora_A, lora_B):
    xf = np.ascontiguousarray(x.reshape(M, I), dtype=np.float32)
    # pack x^T chunks: xt[msb*128 + p, g, m'] = x[msb*MSB + m', g*128 + p]
    xt = (
        xf.reshape(NMSB, MSB, NG, GROUP)
        .transpose(0, 3, 2, 1)
        .astype(ml_dtypes.bfloat16)
        .reshape(NMSB * 128, NG, MSB)
    )
    in_maps = []
    for c in range(N_CORES):
        sl = slice(c * OS, (c + 1) * OS)
        in_maps.append({
            "xt": xt,
            "w": np.ascontiguousarray(
                weight[sl], dtype=np.float32).reshape(OS, NG, GROUP),
            "scl": np.exp(np.ascontiguousarray(
                scale_log.reshape(O, NG)[sl], dtype=np.float32)),
            "la": np.ascontiguousarray(lora_A, dtype=np.float32),
            "lbt": np.ascontiguousarray(lora_B[sl].T, dtype=np.float32),
        })
    return in_maps


def kernel(x, weight, scale_log, codebook, lora_A, lora_B):
    cb0, tk, dk = _codebook_consts(codebook)
    nc = _get_program(cb0, tk, dk)
    in_maps = _prep_in_maps(x, weight, scale_log, lora_A, lora_B)
    res = run_bass_kernel_spmd(nc, in_maps, core_ids=list(range(N_CORES))).results
    out = np.concatenate([res[c]["out"] for c in range(N_CORES)], axis=1)
    return out.reshape(x.shape[0], x.shape[1], O)



# revision 2
# speedup vs baseline: 1.0050x; 1.0050x over previous
"""CodebookLoRASTELinear forward on 8 Trainium2 NeuronCores (v3).

out = x @ (W_q + D)^T with
  D   = (lora_B @ lora_A) * (alpha/rank)
  cb  = codebook / max|codebook|,  S = exp(scale_log)  (per [o, 128] group)
  q   = cb[searchsorted(midpoints(cb), (W+D)/S)],  W_q = q * S

Column-parallel sharding: W / scale / lora_B rows (out_features) split
across 8 cores; x and lora_A replicated; outputs concatenated on host.

v3: phase B runs entirely in TRANSPOSED [i%128, o] layout so the PE
transposes disappear and all quantization thresholds become immediates:
- host passes (W/S)^T; a rank-65 f32r matmul (lora + ones row against
  recipS-scaled lora_B + CC row) emits (D/S + CC)^T per group in one
  213ns instruction; a K=1 ones x S^T f32r matmul broadcasts S^T.
- ACT evacuates both PSUM tiles to SBUF; the whole elementwise chain is
  [128,512]-wide: Pool u/z/weff (tensor_tensor only -- the only legal
  Pool form), DVE a1/a12/a123 (immediate-scalar ops), ACT Sign.
- weff = (q + D/S + CC)*S^T lands directly as bf16 -- no PE transpose,
  no psum cast-copy; phase C is unchanged.
- groups are emitted pair-interleaved (stage-major across 2 groups) so
  per-engine in-order queues pipeline instead of walking one dep chain.
- 6 output-tile chains (chunk 0 + half of chunk 1) overlap phase B.
"""

import numpy as np
import sys

for _p in ("/opt/trn_rl_repo",):
    if _p not in sys.path:
        sys.path.insert(0, _p)

import ml_dtypes  # noqa: E402
import concourse.mybir as mybir  # noqa: E402
import concourse.tile as tile  # noqa: E402
from concourse import bacc  # noqa: E402
from concourse.bass_utils import run_bass_kernel_spmd  # noqa: E402
from contextlib import ExitStack  # noqa: E402

N_CORES = 8
M = 8192  # 4 * 2048 tokens
I = 4096  # in_features
O = 4096  # out_features
GROUP = 128
NG = I // GROUP  # 32 groups along i
RANK = 64
KAUG = RANK + 1  # 65: lora rank + ones row (injects the CC constant)
ALPHA_OVER_RANK = 32.0 / 64.0
OS = O // N_CORES  # 512 out features per core
NOB = OS // 128  # 4 output row blocks per core
MSB = 512  # m columns per x chunk
NMSB = M // MSB  # 16 chunks

F32 = mybir.dt.float32
F32R = mybir.dt.float32r
BF16 = mybir.dt.bfloat16
ALU = mybir.AluOpType
AF = mybir.ActivationFunctionType

_cache = {}


def _build_program(cb0, tk, dk, reps=1, variant=""):
    nc = bacc.Bacc("TRN2", target_bir_lowering=False, debug=False)

    xt_d = nc.dram_tensor("xt", [NMSB * 128, NG, MSB], BF16,
                          kind="ExternalInput").ap()
    # (W/S)^T in per-group row blocks: wnt[g*128 + i, o]
    wnt_d = nc.dram_tensor("wnt", [NG * 128, OS], F32, kind="ExternalInput").ap()
    # S^T broadcast tiles, host-replicated across partitions
    sclt_d = nc.dram_tensor("sclt", [NG * 128, OS], F32,
                            kind="ExternalInput").ap()
    lan_d = nc.dram_tensor("lan", [KAUG, I], F32R, kind="ExternalInput").ap()
    # per-group recipS-scaled lora_B^T (+ CC row): [NG, KAUG, OS]
    lbtn_d = nc.dram_tensor("lbtn", [NG, KAUG, OS], F32R,
                            kind="ExternalInput").ap()
    out_d = nc.dram_tensor("out", [M, OS], F32, kind="ExternalOutput").ap()

    d3_is_one = abs(float(dk[2]) - 1.0) < 1e-12
    CCq = float(cb0) + (float(dk[1]) / 2.0 if d3_is_one else 0.0)
    # u = (W+D)/S + CCq (the lora ones-row injects CCq), so compares use
    # shifted immediates t_k + CCq; z = a123 + dn recovers q + D/S exactly.
    t1, t2, t3 = (float(t) + CCq for t in tk)
    d1, d2, d3 = (float(d) for d in dk)

    with tile.TileContext(nc) as tc, ExitStack() as ctx:
        singles = ctx.enter_context(tc.tile_pool(name="singles", bufs=1))

        # lora factors first -- they gate the PE's first work
        lan_sb = singles.tile([KAUG, I], F32R)
        nc.sync.dma_start(out=lan_sb, in_=lan_d)

        # persistent effective transposed weight, one tile per group
        weff = [
            singles.tile([128, OS], BF16, tag=f"weff{g}", name=f"weff{g}")
            for g in range(NG)
        ]

        if reps > 1:
            ctx.enter_context(tc.For_i(0, reps, 1))

        xpool = ctx.enter_context(tc.tile_pool(name="xpool", bufs=2))
        wload = ctx.enter_context(tc.tile_pool(name="wload", bufs=4))
        lbpool = ctx.enter_context(tc.tile_pool(name="lbpool", bufs=4))
        dnp = ctx.enter_context(tc.tile_pool(name="dnp", bufs=4))
        sbp = ctx.enter_context(tc.tile_pool(name="sbp", bufs=4))
        upool = ctx.enter_context(tc.tile_pool(name="upool", bufs=2))
        a1p = ctx.enter_context(tc.tile_pool(name="a1p", bufs=2))
        a2p = ctx.enter_context(tc.tile_pool(name="a2p", bufs=2))
        a12p = ctx.enter_context(tc.tile_pool(name="a12p", bufs=2))
        a123p = ctx.enter_context(tc.tile_pool(name="a123p", bufs=2))
        zp = ctx.enter_context(tc.tile_pool(name="zp", bufs=2))
        opool = ctx.enter_context(tc.tile_pool(name="opool", bufs=4))
        psumD = ctx.enter_context(tc.tile_pool(name="psumD", bufs=2, space="PSUM"))
        psumO = ctx.enter_context(tc.tile_pool(name="psumO", bufs=1, space="PSUM"))

        def load_chunk(msb, split=1):
            # split>1 loads in k-quads so early chain matmuls only wait
            # for the slice they read
            t = xpool.tile([128, NG, MSB], BF16, tag="xt")
            gq = NG // split
            for q in range(split):
                nc.sync.dma_start(
                    out=t[:, q * gq : (q + 1) * gq, :],
                    in_=xt_d[msb * 128 : (msb + 1) * 128,
                             q * gq : (q + 1) * gq, :])
            return t

        def load_wnt(g):
            t = wload.tile([128, OS], F32, tag="wnt", name=f"wnt{g}")
            nc.sync.dma_start(out=t, in_=wnt_d[g * 128 : (g + 1) * 128, :])
            return t

        def load_lbtn(g):
            t = lbpool.tile([KAUG, OS], F32R, tag="lbtn", name=f"lbtn{g}")
            nc.sync.dma_start(out=t, in_=lbtn_d[g, :, :])
            return t

        wnt_sb = {}
        lbtn_sb = {}

        # 6 output accumulation chains interleaved with phase B
        B_CHAINS = [(0, 0), (0, 1), (0, 2), (0, 3), (1, 0), (1, 1)]
        chainsB = [
            psumO.tile([128, OS], F32, tag=f"o{i}", name=f"chain{i}")
            for i in range(len(B_CHAINS))
        ]
        CH_LAG = [0, 0, 0, 0, 10, 10]  # chunk-1 chains wait for their DMA

        dn_sb = {}  # g -> evacuated (D/S + CC)^T in SBUF
        sb_sb = {}  # g -> S^T broadcast tiles (host-replicated, DMA'd)

        def load_sclb(g):
            t = sbp.tile([128, OS], F32, tag="sb", name=f"sbs{g}")
            nc.sync.dma_start(out=t, in_=sclt_d[g * 128 : (g + 1) * 128, :])
            sb_sb[g] = t

        def emit_mms(g):
            # (D/S + CC)^T for group g: one rank-65 f32r matmul
            d_ps = psumD.tile([128, OS], F32, tag="d", name=f"dn{g}")
            nc.tensor.matmul(
                d_ps, lhsT=lan_sb[:, g * 128 : (g + 1) * 128],
                rhs=lbtn_sb[g], start=True, stop=True,
            )
            return d_ps

        def emit_evac(g, d_ps):
            dn = dnp.tile([128, OS], F32, tag="dn", name=f"dns{g}")
            nc.scalar.copy(dn, d_ps)
            dn_sb[g] = dn

        def emit_elem(gs):
            """Stage-major elementwise for a list of groups (pipelining)."""
            u = {}
            for g in gs:  # u = (W + D)/S + CC   [Pool]
                t = upool.tile([128, OS], F32, tag="u", name=f"u{g}")
                nc.gpsimd.tensor_tensor(t, wnt_sb[g], dn_sb[g], op=ALU.add)
                u[g] = t
            a1 = {}
            for g in gs:  # (u > t1+CC)*d1   [DVE, immediates]
                t = a1p.tile([128, OS], F32, tag="a1", name=f"a1{g}")
                nc.vector.tensor_scalar(t, u[g], t1, d1,
                                        op0=ALU.is_gt, op1=ALU.mult)
                a1[g] = t
            a123 = {}
            if d3_is_one:
                s2 = {}
                for g in gs:  # Sign(u - t2)   [ACT]
                    t = a2p.tile([128, OS], F32, tag="s2", name=f"s2{g}")
                    nc.scalar.activation(t, u[g], AF.Sign, bias=-t2, scale=1.0)
                    s2[g] = t
                a12 = {}
                for g in gs:  # (d2/2)*s2 + a1   [DVE]
                    t = a12p.tile([128, OS], F32, tag="a12", name=f"a12{g}")
                    nc.vector.scalar_tensor_tensor(
                        t, s2[g], d2 / 2.0, a1[g], op0=ALU.mult, op1=ALU.add)
                    a12[g] = t
                for g in gs:  # (u > t3) + a12   [DVE]
                    t = a123p.tile([128, OS], F32, tag="a123", name=f"a123{g}")
                    nc.vector.scalar_tensor_tensor(
                        t, u[g], t3, a12[g], op0=ALU.is_gt, op1=ALU.add)
                    a123[g] = t
            else:
                a2 = {}
                for g in gs:
                    t = a2p.tile([128, OS], F32, tag="s2", name=f"a2{g}")
                    nc.vector.tensor_scalar(t, u[g], t2, d2,
                                            op0=ALU.is_gt, op1=ALU.mult)
                    a2[g] = t
                a12 = {}
                for g in gs:
                    t = a12p.tile([128, OS], F32, tag="a12", name=f"a12{g}")
                    nc.gpsimd.tensor_tensor(t, a1[g], a2[g], op=ALU.add)
                    a12[g] = t
                a3 = {}
                for g in gs:
                    t = a123p.tile([128, OS], F32, tag="a123", name=f"a3{g}")
                    nc.vector.tensor_scalar(t, u[g], t3, d3,
                                            op0=ALU.is_gt, op1=ALU.mult)
                    a3[g] = t
                for g in gs:
                    t = zp.tile([128, OS], F32, tag="z", name=f"a123b{g}")
                    nc.gpsimd.tensor_tensor(t, a12[g], a3[g], op=ALU.add)
                    a123[g] = t
            z = {}
            for g in gs:  # q + D/S + CC_resid = a123 + dn  [Pool]
                t = zp.tile([128, OS], F32, tag="z2", name=f"z{g}")
                nc.gpsimd.tensor_tensor(t, a123[g], dn_sb[g], op=ALU.add)
                z[g] = t
            for g in gs:  # W_eff^T = z * S^T -> bf16   [DVE]
                nc.vector.tensor_tensor(weff[g], z[g], sb_sb[g], op=ALU.mult)
                del dn_sb[g], sb_sb[g]

        def emit_mmB(ci, g):
            msb, mb = B_CHAINS[ci]
            xt_t = xt0 if msb == 0 else xt1
            nc.tensor.matmul(
                chainsB[ci],
                lhsT=xt_t[:, g, mb * 128 : (mb + 1) * 128],
                rhs=weff[g],
                start=(g == 0),
                stop=(g == NG - 1),
                skip_group_check=True,
            )

        # groups 0-3 land before xt0 so production never stalls; xt0
        # arrives just before the first interleaved chain matmul (M_LAG)
        for g in range(4):
            wnt_sb[g] = load_wnt(g)
            lbtn_sb[g] = load_lbtn(g)
            load_sclb(g)
        xt0 = load_chunk(0)
        xt1 = load_chunk(1)

        # pair-wise pipeline: mms+evac for pair p feed elemwise for pair
        # p-1; chains trail M_LAG groups behind production.
        M_LAG = 8
        MAXLAG = M_LAG + max(CH_LAG)
        NP = NG // 2
        for p in range(NP + 1 + (MAXLAG + 1) // 2):
            for h in range(2):
                g = 2 * p + h
                if 4 <= g + 4 < NG:
                    wnt_sb[g + 4] = load_wnt(g + 4)
                    lbtn_sb[g + 4] = load_lbtn(g + 4)
                    load_sclb(g + 4)
                if g < NG:
                    d_ps = emit_mms(g)
                    emit_evac(g, d_ps)
                if h == 1 and 1 <= p <= NP:
                    emit_elem([2 * (p - 1), 2 * (p - 1) + 1])
                for ci in range(len(B_CHAINS)):
                    gg = g - M_LAG - CH_LAG[ci]
                    if 0 <= gg < NG:
                        emit_mmB(ci, gg)

        for ci, (msb, mb) in enumerate(B_CHAINS):
            o_sb = opool.tile([128, OS], F32, tag="o")
            nc.scalar.copy(o_sb, chainsB[ci])
            mbg = msb * NOB + mb
            nc.sync.dma_start(out=out_d[mbg * 128 : (mbg + 1) * 128, :], in_=o_sb)

        # ---- phase C: stream the remaining out-tiles ----
        rest = [(1, 2), (1, 3)] + [(msb, mb) for msb in range(2, NMSB)
                                   for mb in range(NOB)]
        xts = {0: xt0, 1: xt1, 2: load_chunk(2)}
        loaded_upto = 2

        for ti, (msb, mb) in enumerate(rest):
            if mb == 0 and msb + 1 < NMSB and msb + 1 > loaded_upto:
                xts[msb + 1] = load_chunk(msb + 1)
                loaded_upto = msb + 1
            xt_t = xts[msb]
            p_out = psumO.tile([128, OS], F32, tag=f"o{ti % len(B_CHAINS)}")
            for g in range(NG):
                nc.tensor.matmul(
                    p_out,
                    lhsT=xt_t[:, g, mb * 128 : (mb + 1) * 128],
                    rhs=weff[g],
                    start=(g == 0),
                    stop=(g == NG - 1),
                )
            o_sb = opool.tile([128, OS], F32, tag="o")
            nc.scalar.copy(o_sb, p_out)
            mbg = msb * NOB + mb
            nc.sync.dma_start(
                out=out_d[mbg * 128 : (mbg + 1) * 128, :], in_=o_sb
            )

    nc.compile()
    return nc


def _get_program(cb0, tk, dk, reps=1, variant=""):
    key = (round(float(cb0), 9), tuple(round(float(t), 9) for t in tk),
           tuple(round(float(d), 9) for d in dk), reps, variant)
    if key not in _cache:
        _cache[key] = _build_program(cb0, tk, dk, reps, variant)
    return _cache[key]


def _codebook_consts(codebook):
    cb = np.asarray(codebook, dtype=np.float64)
    cb = cb / max(float(np.max(np.abs(cb))), 1e-8)
    tk = (cb[:-1] + cb[1:]) * 0.5
    dk = np.diff(cb)
    return float(cb[0]), [float(v) for v in tk], [float(v) for v in dk]


def _prep_in_maps(x, weight, scale_log, lora_A, lora_B, codebook=None):
    cb0, tk, dk = _codebook_consts(
        codebook if codebook is not None else np.array([-1.0, -0.25, 0.0, 1.0]))
    d3_is_one = abs(float(dk[2]) - 1.0) < 1e-12
    # CC_q shifts the a123 partial sum to the true codebook value; it is
    # injected through the lora ones-row so it never touches the compares.
    CCq = float(cb0) + (float(dk[1]) / 2.0 if d3_is_one else 0.0)

    xf = np.ascontiguousarray(x.reshape(M, I), dtype=np.float32)
    xt = (
        xf.reshape(NMSB, MSB, NG, GROUP)
        .transpose(0, 3, 2, 1)
        .astype(ml_dtypes.bfloat16)
        .reshape(NMSB * 128, NG, MSB)
    )
    scl_full = np.exp(np.ascontiguousarray(
        scale_log.reshape(O, NG), dtype=np.float64))  # [O, NG]
    lan = np.zeros((KAUG, I), dtype=np.float32)
    lan[:RANK] = np.ascontiguousarray(lora_A, dtype=np.float32)
    lan[RANK] = 1.0

    in_maps = []
    for c in range(N_CORES):
        sl = slice(c * OS, (c + 1) * OS)
        w_c = np.asarray(weight[sl], dtype=np.float64)          # [OS, I]
        scl_c = scl_full[sl]                                    # [OS, NG]
        # (W/S)^T per-group row blocks
        wns = (w_c.reshape(OS, NG, GROUP) / scl_c[:, :, None])  # [OS, NG, G]
        wnt = np.ascontiguousarray(
            wns.transpose(1, 2, 0).reshape(NG * GROUP, OS)).astype(np.float32)
        # recipS-scaled lora_B^T + CC row, per group
        lbt = (np.asarray(lora_B[sl], dtype=np.float64).T
               * ALPHA_OVER_RANK)                               # [RANK, OS]
        lbtn = np.empty((NG, KAUG, OS), dtype=np.float32)
        for g in range(NG):
            lbtn[g, :RANK] = (lbt / scl_c[:, g][None, :]).astype(np.float32)
            lbtn[g, RANK] = CCq
        in_maps.append({
            "xt": xt,
            "wnt": wnt,
            "sclt": np.ascontiguousarray(np.repeat(
                scl_c.T.astype(np.float32)[:, None, :], 128, axis=1
            ).reshape(NG * 128, OS)),
            "lan": lan,
            "lbtn": lbtn,
        })
    return in_maps


def kernel(x, weight, scale_log, codebook, lora_A, lora_B):
    cb0, tk, dk = _codebook_consts(codebook)
    nc = _get_program(cb0, tk, dk)
    in_maps = _prep_in_maps(x, weight, scale_log, lora_A, lora_B, codebook)
    res = run_bass_kernel_spmd(nc, in_maps, core_ids=list(range(N_CORES))).results
    out = np.concatenate([res[c]["out"] for c in range(N_CORES)], axis=1)
    return out.reshape(x.shape[0], x.shape[1], O)


# revision 3
# speedup vs baseline: 1.0175x; 1.0124x over previous
"""CodebookLoRASTELinear forward on 8 Trainium2 NeuronCores (v3).

out = x @ (W_q + D)^T with
  D   = (lora_B @ lora_A) * (alpha/rank)
  cb  = codebook / max|codebook|,  S = exp(scale_log)  (per [o, 128] group)
  q   = cb[searchsorted(midpoints(cb), (W+D)/S)],  W_q = q * S

Column-parallel sharding: W / scale / lora_B rows (out_features) split
across 8 cores; x and lora_A replicated; outputs concatenated on host.

v3: phase B runs entirely in TRANSPOSED [i%128, o] layout so the PE
transposes disappear and all quantization thresholds become immediates:
- host passes (W/S)^T; a rank-65 f32r matmul (lora + ones row against
  recipS-scaled lora_B + CC row) emits (D/S + CC)^T per group in one
  213ns instruction; a K=1 ones x S^T f32r matmul broadcasts S^T.
- ACT evacuates both PSUM tiles to SBUF; the whole elementwise chain is
  [128,512]-wide: Pool u/z/weff (tensor_tensor only -- the only legal
  Pool form), DVE a1/a12/a123 (immediate-scalar ops), ACT Sign.
- weff = (q + D/S + CC)*S^T lands directly as bf16 -- no PE transpose,
  no psum cast-copy; phase C is unchanged.
- groups are emitted pair-interleaved (stage-major across 2 groups) so
  per-engine in-order queues pipeline instead of walking one dep chain.
- 6 output-tile chains (chunk 0 + half of chunk 1) overlap phase B.
"""

import numpy as np
import sys

for _p in ("/opt/trn_rl_repo",):
    if _p not in sys.path:
        sys.path.insert(0, _p)

import ml_dtypes  # noqa: E402
import concourse.mybir as mybir  # noqa: E402
import concourse.tile as tile  # noqa: E402
from concourse import bacc  # noqa: E402
from concourse.bass_utils import run_bass_kernel_spmd  # noqa: E402
from contextlib import ExitStack  # noqa: E402

N_CORES = 8
M = 8192  # 4 * 2048 tokens
I = 4096  # in_features
O = 4096  # out_features
GROUP = 128
NG = I // GROUP  # 32 groups along i
RANK = 64
KAUG = RANK + 1  # 65: lora rank + ones row (injects the CC constant)
ALPHA_OVER_RANK = 32.0 / 64.0
OS = O // N_CORES  # 512 out features per core
NOB = OS // 128  # 4 output row blocks per core
MSB = 512  # m columns per x chunk
NMSB = M // MSB  # 16 chunks

F32 = mybir.dt.float32
F32R = mybir.dt.float32r
BF16 = mybir.dt.bfloat16
ALU = mybir.AluOpType
AF = mybir.ActivationFunctionType

_cache = {}


def _build_program(cb0, tk, dk, reps=1, variant=""):
    nc = bacc.Bacc("TRN2", target_bir_lowering=False, debug=False)

    xt_d = nc.dram_tensor("xt", [NMSB * 128, NG, MSB], BF16,
                          kind="ExternalInput").ap()
    # (W/S)^T in per-group row blocks: wnt[g*128 + i, o]
    wnt_d = nc.dram_tensor("wnt", [NG * 128, OS], F32, kind="ExternalInput").ap()
    # S^T broadcast tiles, host-replicated across partitions
    sclt_d = nc.dram_tensor("sclt", [NG * 128, OS], F32,
                            kind="ExternalInput").ap()
    lan_d = nc.dram_tensor("lan", [KAUG, I], F32R, kind="ExternalInput").ap()
    # per-group recipS-scaled lora_B^T (+ CC row): [NG, KAUG, OS]
    lbtn_d = nc.dram_tensor("lbtn", [NG, KAUG, OS], F32R,
                            kind="ExternalInput").ap()
    out_d = nc.dram_tensor("out", [M, OS], F32, kind="ExternalOutput").ap()

    d3_is_one = abs(float(dk[2]) - 1.0) < 1e-12
    CCq = float(cb0) + (float(dk[1]) / 2.0 if d3_is_one else 0.0)
    # u = (W+D)/S + CCq (the lora ones-row injects CCq), so compares use
    # shifted immediates t_k + CCq; z = a123 + dn recovers q + D/S exactly.
    t1, t2, t3 = (float(t) + CCq for t in tk)
    d1, d2, d3 = (float(d) for d in dk)

    with tile.TileContext(nc) as tc, ExitStack() as ctx:
        singles = ctx.enter_context(tc.tile_pool(name="singles", bufs=1))

        # lora factors first -- they gate the PE's first work
        lan_sb = singles.tile([KAUG, I], F32R)
        nc.sync.dma_start(out=lan_sb, in_=lan_d)

        # persistent effective transposed weight, one tile per group
        weff = [
            singles.tile([128, OS], BF16, tag=f"weff{g}", name=f"weff{g}")
            for g in range(NG)
        ]

        if reps > 1:
            ctx.enter_context(tc.For_i(0, reps, 1))

        xpool = ctx.enter_context(tc.tile_pool(name="xpool", bufs=2))
        wload = ctx.enter_context(tc.tile_pool(name="wload", bufs=4))
        lbpool = ctx.enter_context(tc.tile_pool(name="lbpool", bufs=4))
        dnp = ctx.enter_context(tc.tile_pool(name="dnp", bufs=4))
        sbp = ctx.enter_context(tc.tile_pool(name="sbp", bufs=4))
        upool = ctx.enter_context(tc.tile_pool(name="upool", bufs=2))
        a1p = ctx.enter_context(tc.tile_pool(name="a1p", bufs=2))
        a2p = ctx.enter_context(tc.tile_pool(name="a2p", bufs=2))
        a12p = ctx.enter_context(tc.tile_pool(name="a12p", bufs=2))
        a123p = ctx.enter_context(tc.tile_pool(name="a123p", bufs=2))
        zp = ctx.enter_context(tc.tile_pool(name="zp", bufs=2))
        opool = ctx.enter_context(tc.tile_pool(name="opool", bufs=4))
        psumD = ctx.enter_context(tc.tile_pool(name="psumD", bufs=2, space="PSUM"))
        psumO = ctx.enter_context(tc.tile_pool(name="psumO", bufs=1, space="PSUM"))

        def load_chunk(msb, split=1):
            # split>1 loads in k-quads so early chain matmuls only wait
            # for the slice they read
            t = xpool.tile([128, NG, MSB], BF16, tag="xt")
            gq = NG // split
            for q in range(split):
                nc.scalar.dma_start(
                    out=t[:, q * gq : (q + 1) * gq, :],
                    in_=xt_d[msb * 128 : (msb + 1) * 128,
                             q * gq : (q + 1) * gq, :])
            return t

        def load_wnt(g):
            t = wload.tile([128, OS], F32, tag="wnt", name=f"wnt{g}")
            nc.sync.dma_start(out=t, in_=wnt_d[g * 128 : (g + 1) * 128, :])
            return t

        def load_lbtn(g):
            t = lbpool.tile([KAUG, OS], F32R, tag="lbtn", name=f"lbtn{g}")
            nc.sync.dma_start(out=t, in_=lbtn_d[g, :, :])
            return t

        wnt_sb = {}
        lbtn_sb = {}

        # 6 output accumulation chains interleaved with phase B
        B_CHAINS = [(0, 0), (0, 1), (0, 2), (0, 3), (1, 0), (1, 1)]
        chainsB = [
            psumO.tile([128, OS], F32, tag=f"o{i}", name=f"chain{i}")
            for i in range(len(B_CHAINS))
        ]
        CH_LAG = [0, 0, 0, 0, 10, 10]  # chunk-1 chains wait for their DMA

        dn_sb = {}  # g -> evacuated (D/S + CC)^T in SBUF
        sb_sb = {}  # g -> S^T broadcast tiles (host-replicated, DMA'd)

        def load_sclb(g):
            t = sbp.tile([128, OS], F32, tag="sb", name=f"sbs{g}")
            nc.sync.dma_start(out=t, in_=sclt_d[g * 128 : (g + 1) * 128, :])
            sb_sb[g] = t

        def emit_mms(g):
            # (D/S + CC)^T for group g: one rank-65 f32r matmul
            d_ps = psumD.tile([128, OS], F32, tag="d", name=f"dn{g}")
            nc.tensor.matmul(
                d_ps, lhsT=lan_sb[:, g * 128 : (g + 1) * 128],
                rhs=lbtn_sb[g], start=True, stop=True,
            )
            return d_ps

        def emit_evac(g, d_ps):
            dn = dnp.tile([128, OS], F32, tag="dn", name=f"dns{g}")
            nc.scalar.copy(dn, d_ps)
            dn_sb[g] = dn

        def emit_elem(gs):
            """Stage-major elementwise for a list of groups (pipelining)."""
            u = {}
            for g in gs:  # u = (W + D)/S + CC   [Pool]
                t = upool.tile([128, OS], F32, tag="u", name=f"u{g}")
                nc.gpsimd.tensor_tensor(t, wnt_sb[g], dn_sb[g], op=ALU.add)
                u[g] = t
            a1 = {}
            for g in gs:  # (u > t1+CC)*d1   [DVE, immediates]
                t = a1p.tile([128, OS], F32, tag="a1", name=f"a1{g}")
                nc.vector.tensor_scalar(t, u[g], t1, d1,
                                        op0=ALU.is_gt, op1=ALU.mult)
                a1[g] = t
            a123 = {}
            if d3_is_one:
                s2 = {}
                for g in gs:  # Sign(u - t2)   [ACT]
                    t = a2p.tile([128, OS], F32, tag="s2", name=f"s2{g}")
                    nc.scalar.activation(t, u[g], AF.Sign, bias=-t2, scale=1.0)
                    s2[g] = t
                a12 = {}
                for g in gs:  # (d2/2)*s2 + a1   [DVE]
                    t = a12p.tile([128, OS], F32, tag="a12", name=f"a12{g}")
                    nc.vector.scalar_tensor_tensor(
                        t, s2[g], d2 / 2.0, a1[g], op0=ALU.mult, op1=ALU.add)
                    a12[g] = t
                for g in gs:  # (u > t3) + a12   [DVE]
                    t = a123p.tile([128, OS], F32, tag="a123", name=f"a123{g}")
                    nc.vector.scalar_tensor_tensor(
                        t, u[g], t3, a12[g], op0=ALU.is_gt, op1=ALU.add)
                    a123[g] = t
            else:
                a2 = {}
                for g in gs:
                    t = a2p.tile([128, OS], F32, tag="s2", name=f"a2{g}")
                    nc.vector.tensor_scalar(t, u[g], t2, d2,
                                            op0=ALU.is_gt, op1=ALU.mult)
                    a2[g] = t
                a12 = {}
                for g in gs:
                    t = a12p.tile([128, OS], F32, tag="a12", name=f"a12{g}")
                    nc.gpsimd.tensor_tensor(t, a1[g], a2[g], op=ALU.add)
                    a12[g] = t
                a3 = {}
                for g in gs:
                    t = a123p.tile([128, OS], F32, tag="a123", name=f"a3{g}")
                    nc.vector.tensor_scalar(t, u[g], t3, d3,
                                            op0=ALU.is_gt, op1=ALU.mult)
                    a3[g] = t
                for g in gs:
                    t = zp.tile([128, OS], F32, tag="z", name=f"a123b{g}")
                    nc.gpsimd.tensor_tensor(t, a12[g], a3[g], op=ALU.add)
                    a123[g] = t
            z = {}
            for g in gs:  # q + D/S + CC_resid = a123 + dn  [Pool]
                t = zp.tile([128, OS], F32, tag="z2", name=f"z{g}")
                nc.gpsimd.tensor_tensor(t, a123[g], dn_sb[g], op=ALU.add)
                z[g] = t
            for g in gs:  # W_eff^T = z * S^T -> bf16   [DVE]
                nc.vector.tensor_tensor(weff[g], z[g], sb_sb[g], op=ALU.mult)
                del dn_sb[g], sb_sb[g]

        def emit_mmB(ci, g):
            msb, mb = B_CHAINS[ci]
            xt_t = xt0 if msb == 0 else xt1
            nc.tensor.matmul(
                chainsB[ci],
                lhsT=xt_t[:, g, mb * 128 : (mb + 1) * 128],
                rhs=weff[g],
                start=(g == 0),
                stop=(g == NG - 1),
                skip_group_check=True,
            )

        # groups 0-3 land before xt0 so production never stalls; xt0
        # arrives just before the first interleaved chain matmul (M_LAG)
        for g in range(4):
            wnt_sb[g] = load_wnt(g)
            lbtn_sb[g] = load_lbtn(g)
            load_sclb(g)
        xt0 = load_chunk(0)
        xt1 = load_chunk(1)

        # pair-wise pipeline: mms+evac for pair p feed elemwise for pair
        # p-1; chains trail M_LAG groups behind production.
        M_LAG = 8
        MAXLAG = M_LAG + max(CH_LAG)
        NP = NG // 2
        for p in range(NP + 1 + (MAXLAG + 1) // 2):
            for h in range(2):
                g = 2 * p + h
                if 4 <= g + 4 < NG:
                    wnt_sb[g + 4] = load_wnt(g + 4)
                    lbtn_sb[g + 4] = load_lbtn(g + 4)
                    load_sclb(g + 4)
                if g < NG:
                    d_ps = emit_mms(g)
                    emit_evac(g, d_ps)
                if h == 1 and 1 <= p <= NP:
                    emit_elem([2 * (p - 1), 2 * (p - 1) + 1])
                for ci in range(len(B_CHAINS)):
                    gg = g - M_LAG - CH_LAG[ci]
                    if 0 <= gg < NG:
                        emit_mmB(ci, gg)

        for ci, (msb, mb) in enumerate(B_CHAINS):
            o_sb = opool.tile([128, OS], F32, tag="o")
            nc.scalar.copy(o_sb, chainsB[ci])
            mbg = msb * NOB + mb
            nc.sync.dma_start(out=out_d[mbg * 128 : (mbg + 1) * 128, :], in_=o_sb)

        # ---- phase C: stream the remaining out-tiles ----
        rest = [(1, 2), (1, 3)] + [(msb, mb) for msb in range(2, NMSB)
                                   for mb in range(NOB)]
        xts = {0: xt0, 1: xt1, 2: load_chunk(2)}
        loaded_upto = 2

        for ti, (msb, mb) in enumerate(rest):
            if mb == 0 and msb + 1 < NMSB and msb + 1 > loaded_upto:
                xts[msb + 1] = load_chunk(msb + 1)
                loaded_upto = msb + 1
            xt_t = xts[msb]
            p_out = psumO.tile([128, OS], F32, tag=f"o{ti % len(B_CHAINS)}")
            for g in range(NG):
                nc.tensor.matmul(
                    p_out,
                    lhsT=xt_t[:, g, mb * 128 : (mb + 1) * 128],
                    rhs=weff[g],
                    start=(g == 0),
                    stop=(g == NG - 1),
                )
            o_sb = opool.tile([128, OS], F32, tag="o")
            nc.scalar.copy(o_sb, p_out)
            mbg = msb * NOB + mb
            nc.sync.dma_start(
                out=out_d[mbg * 128 : (mbg + 1) * 128, :], in_=o_sb
            )

    nc.compile()
    return nc


def _get_program(cb0, tk, dk, reps=1, variant=""):
    key = (round(float(cb0), 9), tuple(round(float(t), 9) for t in tk),
           tuple(round(float(d), 9) for d in dk), reps, variant)
    if key not in _cache:
        _cache[key] = _build_program(cb0, tk, dk, reps, variant)
    return _cache[key]


def _codebook_consts(codebook):
    cb = np.asarray(codebook, dtype=np.float64)
    cb = cb / max(float(np.max(np.abs(cb))), 1e-8)
    tk = (cb[:-1] + cb[1:]) * 0.5
    dk = np.diff(cb)
    return float(cb[0]), [float(v) for v in tk], [float(v) for v in dk]


def _prep_in_maps(x, weight, scale_log, lora_A, lora_B, codebook=None):
    cb0, tk, dk = _codebook_consts(
        codebook if codebook is not None else np.array([-1.0, -0.25, 0.0, 1.0]))
    d3_is_one = abs(float(dk[2]) - 1.0) < 1e-12
    # CC_q shifts the a123 partial sum to the true codebook value; it is
    # injected through the lora ones-row so it never touches the compares.
    CCq = float(cb0) + (float(dk[1]) / 2.0 if d3_is_one else 0.0)

    xf = np.ascontiguousarray(x.reshape(M, I), dtype=np.float32)
    xt = (
        xf.reshape(NMSB, MSB, NG, GROUP)
        .transpose(0, 3, 2, 1)
        .astype(ml_dtypes.bfloat16)
        .reshape(NMSB * 128, NG, MSB)
    )
    scl_full = np.exp(np.ascontiguousarray(
        scale_log.reshape(O, NG), dtype=np.float64))  # [O, NG]
    lan = np.zeros((KAUG, I), dtype=np.float32)
    lan[:RANK] = np.ascontiguousarray(lora_A, dtype=np.float32)
    lan[RANK] = 1.0

    in_maps = []
    for c in range(N_CORES):
        sl = slice(c * OS, (c + 1) * OS)
        w_c = np.asarray(weight[sl], dtype=np.float64)          # [OS, I]
        scl_c = scl_full[sl]                                    # [OS, NG]
        # (W/S)^T per-group row blocks
        wns = (w_c.reshape(OS, NG, GROUP) / scl_c[:, :, None])  # [OS, NG, G]
        wnt = np.ascontiguousarray(
            wns.transpose(1, 2, 0).reshape(NG * GROUP, OS)).astype(np.float32)
        # recipS-scaled lora_B^T + CC row, per group
        lbt = (np.asarray(lora_B[sl], dtype=np.float64).T
               * ALPHA_OVER_RANK)                               # [RANK, OS]
        lbtn = np.empty((NG, KAUG, OS), dtype=np.float32)
        for g in range(NG):
            lbtn[g, :RANK] = (lbt / scl_c[:, g][None, :]).astype(np.float32)
            lbtn[g, RANK] = CCq
        in_maps.append({
            "xt": xt,
            "wnt": wnt,
            "sclt": np.ascontiguousarray(np.repeat(
                scl_c.T.astype(np.float32)[:, None, :], 128, axis=1
            ).reshape(NG * 128, OS)),
            "lan": lan,
            "lbtn": lbtn,
        })
    return in_maps


def kernel(x, weight, scale_log, codebook, lora_A, lora_B):
    cb0, tk, dk = _codebook_consts(codebook)
    nc = _get_program(cb0, tk, dk)
    in_maps = _prep_in_maps(x, weight, scale_log, lora_A, lora_B, codebook)
    res = run_bass_kernel_spmd(nc, in_maps, core_ids=list(range(N_CORES))).results
    out = np.concatenate([res[c]["out"] for c in range(N_CORES)], axis=1)
    return out.reshape(x.shape[0], x.shape[1], O)


# revision 4
# speedup vs baseline: 1.0494x; 1.0314x over previous
"""CodebookLoRASTELinear forward on 8 Trainium2 NeuronCores (v3).

out = x @ (W_q + D)^T with
  D   = (lora_B @ lora_A) * (alpha/rank)
  cb  = codebook / max|codebook|,  S = exp(scale_log)  (per [o, 128] group)
  q   = cb[searchsorted(midpoints(cb), (W+D)/S)],  W_q = q * S

Column-parallel sharding: W / scale / lora_B rows (out_features) split
across 8 cores; x and lora_A replicated; outputs concatenated on host.

v3: phase B runs entirely in TRANSPOSED [i%128, o] layout so the PE
transposes disappear and all quantization thresholds become immediates:
- host passes (W/S)^T; a rank-65 f32r matmul (lora + ones row against
  recipS-scaled lora_B + CC row) emits (D/S + CC)^T per group in one
  213ns instruction; a K=1 ones x S^T f32r matmul broadcasts S^T.
- ACT evacuates both PSUM tiles to SBUF; the whole elementwise chain is
  [128,512]-wide: Pool u/z/weff (tensor_tensor only -- the only legal
  Pool form), DVE a1/a12/a123 (immediate-scalar ops), ACT Sign.
- weff = (q + D/S + CC)*S^T lands directly as bf16 -- no PE transpose,
  no psum cast-copy; phase C is unchanged.
- groups are emitted pair-interleaved (stage-major across 2 groups) so
  per-engine in-order queues pipeline instead of walking one dep chain.
- 6 output-tile chains (chunk 0 + half of chunk 1) overlap phase B.
"""

import numpy as np
import sys

for _p in ("/opt/trn_rl_repo",):
    if _p not in sys.path:
        sys.path.insert(0, _p)

import ml_dtypes  # noqa: E402
import concourse.mybir as mybir  # noqa: E402
import concourse.tile as tile  # noqa: E402
from concourse import bacc  # noqa: E402
from concourse.bass_utils import run_bass_kernel_spmd  # noqa: E402
from contextlib import ExitStack  # noqa: E402

N_CORES = 8
M = 8192  # 4 * 2048 tokens
I = 4096  # in_features
O = 4096  # out_features
GROUP = 128
NG = I // GROUP  # 32 groups along i
RANK = 64
KAUG = RANK + 1  # 65: lora rank + ones row (injects the CC constant)
ALPHA_OVER_RANK = 32.0 / 64.0
OS = O // N_CORES  # 512 out features per core
NOB = OS // 128  # 4 output row blocks per core
MSB = 512  # m columns per x chunk
NMSB = M // MSB  # 16 chunks

F32 = mybir.dt.float32
F32R = mybir.dt.float32r
BF16 = mybir.dt.bfloat16
ALU = mybir.AluOpType
AF = mybir.ActivationFunctionType

_cache = {}


def _build_program(cb0, tk, dk, reps=1, variant=""):
    nc = bacc.Bacc("TRN2", target_bir_lowering=False, debug=False)

    xt_d = nc.dram_tensor("xt", [NMSB * 128, NG, MSB], BF16,
                          kind="ExternalInput").ap()
    # (W/S)^T in per-group row blocks: wnt[g*128 + i, o]
    wnt_d = nc.dram_tensor("wnt", [NG * 128, OS], F32, kind="ExternalInput").ap()
    # S^T broadcast tiles, host-replicated across partitions
    sclt_d = nc.dram_tensor("sclt", [NG * 128, OS], BF16,
                            kind="ExternalInput").ap()
    lan_d = nc.dram_tensor("lan", [KAUG, I], F32R, kind="ExternalInput").ap()
    # per-group recipS-scaled lora_B^T (+ CC row): [NG, KAUG, OS]
    lbtn_d = nc.dram_tensor("lbtn", [NG, KAUG, OS], F32R,
                            kind="ExternalInput").ap()
    out_d = nc.dram_tensor("out", [M, OS], BF16, kind="ExternalOutput").ap()

    d3_is_one = abs(float(dk[2]) - 1.0) < 1e-12
    CCq = float(cb0) + (float(dk[1]) / 2.0 if d3_is_one else 0.0)
    # u = (W+D)/S + CCq (the lora ones-row injects CCq), so compares use
    # shifted immediates t_k + CCq; z = a123 + dn recovers q + D/S exactly.
    t1, t2, t3 = (float(t) + CCq for t in tk)
    d1, d2, d3 = (float(d) for d in dk)

    with tile.TileContext(nc) as tc, ExitStack() as ctx:
        singles = ctx.enter_context(tc.tile_pool(name="singles", bufs=1))

        # lora factors first -- they gate the PE's first work
        lan_sb = singles.tile([KAUG, I], F32R)
        nc.sync.dma_start(out=lan_sb, in_=lan_d)

        # persistent effective transposed weight, one tile per group
        weff = [
            singles.tile([128, OS], BF16, tag=f"weff{g}", name=f"weff{g}")
            for g in range(NG)
        ]

        if reps > 1:
            ctx.enter_context(tc.For_i(0, reps, 1))

        xpool = ctx.enter_context(tc.tile_pool(name="xpool", bufs=2))
        wload = ctx.enter_context(tc.tile_pool(name="wload", bufs=4))
        lbpool = ctx.enter_context(tc.tile_pool(name="lbpool", bufs=4))
        dnp = ctx.enter_context(tc.tile_pool(name="dnp", bufs=4))
        sbp = ctx.enter_context(tc.tile_pool(name="sbp", bufs=4))
        upool = ctx.enter_context(tc.tile_pool(name="upool", bufs=2))
        a1p = ctx.enter_context(tc.tile_pool(name="a1p", bufs=2))
        a2p = ctx.enter_context(tc.tile_pool(name="a2p", bufs=2))
        a12p = ctx.enter_context(tc.tile_pool(name="a12p", bufs=2))
        a123p = ctx.enter_context(tc.tile_pool(name="a123p", bufs=2))
        zp = ctx.enter_context(tc.tile_pool(name="zp", bufs=2))
        opool = ctx.enter_context(tc.tile_pool(name="opool", bufs=4))
        psumD = ctx.enter_context(tc.tile_pool(name="psumD", bufs=2, space="PSUM"))
        psumO = ctx.enter_context(tc.tile_pool(name="psumO", bufs=1, space="PSUM"))

        def load_chunk(msb, split=1):
            # split>1 loads in k-quads so early chain matmuls only wait
            # for the slice they read
            t = xpool.tile([128, NG, MSB], BF16, tag="xt")
            gq = NG // split
            for q in range(split):
                nc.scalar.dma_start(
                    out=t[:, q * gq : (q + 1) * gq, :],
                    in_=xt_d[msb * 128 : (msb + 1) * 128,
                             q * gq : (q + 1) * gq, :])
            return t

        def load_wnt(g):
            t = wload.tile([128, OS], F32, tag="wnt", name=f"wnt{g}")
            nc.sync.dma_start(out=t, in_=wnt_d[g * 128 : (g + 1) * 128, :])
            return t

        def load_lbtn(g):
            t = lbpool.tile([KAUG, OS], F32R, tag="lbtn", name=f"lbtn{g}")
            nc.sync.dma_start(out=t, in_=lbtn_d[g, :, :])
            return t

        wnt_sb = {}
        lbtn_sb = {}

        # 6 output accumulation chains interleaved with phase B
        B_CHAINS = [(0, 0), (0, 1), (0, 2), (0, 3), (1, 0), (1, 1)]
        chainsB = [
            psumO.tile([128, OS], F32, tag=f"o{i}", name=f"chain{i}")
            for i in range(len(B_CHAINS))
        ]
        CH_LAG = [0, 0, 0, 0, 10, 10]  # chunk-1 chains wait for their DMA

        dn_sb = {}  # g -> evacuated (D/S + CC)^T in SBUF
        sb_sb = {}  # g -> S^T broadcast tiles (host-replicated, DMA'd)

        def load_sclb(g):
            t = sbp.tile([128, OS], BF16, tag="sb", name=f"sbs{g}")
            nc.sync.dma_start(out=t, in_=sclt_d[g * 128 : (g + 1) * 128, :])
            sb_sb[g] = t

        def emit_mms(g):
            # (D/S + CC)^T for group g: one rank-65 f32r matmul
            d_ps = psumD.tile([128, OS], F32, tag="d", name=f"dn{g}")
            nc.tensor.matmul(
                d_ps, lhsT=lan_sb[:, g * 128 : (g + 1) * 128],
                rhs=lbtn_sb[g], start=True, stop=True,
            )
            return d_ps

        def emit_evac(g, d_ps):
            dn = dnp.tile([128, OS], F32, tag="dn", name=f"dns{g}")
            nc.scalar.copy(dn, d_ps)
            dn_sb[g] = dn

        def emit_elem(gs):
            """Stage-major elementwise for a list of groups (pipelining)."""
            u = {}
            for g in gs:  # u = (W + D)/S + CC   [Pool]
                t = upool.tile([128, OS], F32, tag="u", name=f"u{g}")
                nc.gpsimd.tensor_tensor(t, wnt_sb[g], dn_sb[g], op=ALU.add)
                u[g] = t
            a1 = {}
            for g in gs:  # (u > t1+CC)*d1   [DVE, immediates]
                t = a1p.tile([128, OS], F32, tag="a1", name=f"a1{g}")
                nc.vector.tensor_scalar(t, u[g], t1, d1,
                                        op0=ALU.is_gt, op1=ALU.mult)
                a1[g] = t
            a123 = {}
            if d3_is_one:
                s2 = {}
                for g in gs:  # Sign(u - t2)   [ACT]
                    t = a2p.tile([128, OS], F32, tag="s2", name=f"s2{g}")
                    nc.scalar.activation(t, u[g], AF.Sign, bias=-t2, scale=1.0)
                    s2[g] = t
                a12 = {}
                for g in gs:  # (d2/2)*s2 + a1   [DVE]
                    t = a12p.tile([128, OS], F32, tag="a12", name=f"a12{g}")
                    nc.vector.scalar_tensor_tensor(
                        t, s2[g], d2 / 2.0, a1[g], op0=ALU.mult, op1=ALU.add)
                    a12[g] = t
                for g in gs:  # (u > t3) + a12   [DVE]
                    t = a123p.tile([128, OS], F32, tag="a123", name=f"a123{g}")
                    nc.vector.scalar_tensor_tensor(
                        t, u[g], t3, a12[g], op0=ALU.is_gt, op1=ALU.add)
                    a123[g] = t
            else:
                a2 = {}
                for g in gs:
                    t = a2p.tile([128, OS], F32, tag="s2", name=f"a2{g}")
                    nc.vector.tensor_scalar(t, u[g], t2, d2,
                                            op0=ALU.is_gt, op1=ALU.mult)
                    a2[g] = t
                a12 = {}
                for g in gs:
                    t = a12p.tile([128, OS], F32, tag="a12", name=f"a12{g}")
                    nc.gpsimd.tensor_tensor(t, a1[g], a2[g], op=ALU.add)
                    a12[g] = t
                a3 = {}
                for g in gs:
                    t = a123p.tile([128, OS], F32, tag="a123", name=f"a3{g}")
                    nc.vector.tensor_scalar(t, u[g], t3, d3,
                                            op0=ALU.is_gt, op1=ALU.mult)
                    a3[g] = t
                for g in gs:
                    t = zp.tile([128, OS], F32, tag="z", name=f"a123b{g}")
                    nc.gpsimd.tensor_tensor(t, a12[g], a3[g], op=ALU.add)
                    a123[g] = t
            z = {}
            for g in gs:  # q + D/S + CC_resid = a123 + dn  [Pool]
                t = zp.tile([128, OS], F32, tag="z2", name=f"z{g}")
                nc.gpsimd.tensor_tensor(t, a123[g], dn_sb[g], op=ALU.add)
                z[g] = t
            for g in gs:  # W_eff^T = z * S^T -> bf16   [DVE]
                nc.vector.tensor_tensor(weff[g], z[g], sb_sb[g], op=ALU.mult)
                del dn_sb[g], sb_sb[g]

        def emit_mmB(ci, g):
            msb, mb = B_CHAINS[ci]
            xt_t = xt0 if msb == 0 else xt1
            nc.tensor.matmul(
                chainsB[ci],
                lhsT=xt_t[:, g, mb * 128 : (mb + 1) * 128],
                rhs=weff[g],
                start=(g == 0),
                stop=(g == NG - 1),
                skip_group_check=True,
            )

        # groups 0-3 land before xt0 so production never stalls; xt0
        # arrives just before the first interleaved chain matmul (M_LAG)
        for g in range(4):
            wnt_sb[g] = load_wnt(g)
            lbtn_sb[g] = load_lbtn(g)
            load_sclb(g)
        xt0 = load_chunk(0)
        xt1 = load_chunk(1)

        # pair-wise pipeline: mms+evac for pair p feed elemwise for pair
        # p-1; chains trail M_LAG groups behind production.
        M_LAG = 8
        MAXLAG = M_LAG + max(CH_LAG)
        NP = NG // 2
        for p in range(NP + 1 + (MAXLAG + 1) // 2):
            for h in range(2):
                g = 2 * p + h
                if 4 <= g + 4 < NG:
                    wnt_sb[g + 4] = load_wnt(g + 4)
                    lbtn_sb[g + 4] = load_lbtn(g + 4)
                    load_sclb(g + 4)
                if g < NG:
                    d_ps = emit_mms(g)
                    emit_evac(g, d_ps)
                if h == 1 and 1 <= p <= NP:
                    emit_elem([2 * (p - 1), 2 * (p - 1) + 1])
                for ci in range(len(B_CHAINS)):
                    gg = g - M_LAG - CH_LAG[ci]
                    if 0 <= gg < NG:
                        emit_mmB(ci, gg)

        for ci, (msb, mb) in enumerate(B_CHAINS):
            o_sb = opool.tile([128, OS], BF16, tag="o")
            nc.scalar.copy(o_sb, chainsB[ci])
            mbg = msb * NOB + mb
            nc.sync.dma_start(out=out_d[mbg * 128 : (mbg + 1) * 128, :], in_=o_sb)

        # ---- phase C: stream the remaining out-tiles ----
        rest = [(1, 2), (1, 3)] + [(msb, mb) for msb in range(2, NMSB)
                                   for mb in range(NOB)]
        xts = {0: xt0, 1: xt1, 2: load_chunk(2)}
        loaded_upto = 2

        for ti, (msb, mb) in enumerate(rest):
            if mb == 0 and msb + 1 < NMSB and msb + 1 > loaded_upto:
                xts[msb + 1] = load_chunk(msb + 1)
                loaded_upto = msb + 1
            xt_t = xts[msb]
            p_out = psumO.tile([128, OS], F32, tag=f"o{ti % len(B_CHAINS)}")
            for g in range(NG):
                nc.tensor.matmul(
                    p_out,
                    lhsT=xt_t[:, g, mb * 128 : (mb + 1) * 128],
                    rhs=weff[g],
                    start=(g == 0),
                    stop=(g == NG - 1),
                )
            o_sb = opool.tile([128, OS], BF16, tag="o")
            nc.scalar.copy(o_sb, p_out)
            mbg = msb * NOB + mb
            nc.sync.dma_start(
                out=out_d[mbg * 128 : (mbg + 1) * 128, :], in_=o_sb
            )

    nc.compile()
    return nc


def _get_program(cb0, tk, dk, reps=1, variant=""):
    key = (round(float(cb0), 9), tuple(round(float(t), 9) for t in tk),
           tuple(round(float(d), 9) for d in dk), reps, variant)
    if key not in _cache:
        _cache[key] = _build_program(cb0, tk, dk, reps, variant)
    return _cache[key]


def _codebook_consts(codebook):
    cb = np.asarray(codebook, dtype=np.float64)
    cb = cb / max(float(np.max(np.abs(cb))), 1e-8)
    tk = (cb[:-1] + cb[1:]) * 0.5
    dk = np.diff(cb)
    return float(cb[0]), [float(v) for v in tk], [float(v) for v in dk]


def _prep_in_maps(x, weight, scale_log, lora_A, lora_B, codebook=None):
    cb0, tk, dk = _codebook_consts(
        codebook if codebook is not None else np.array([-1.0, -0.25, 0.0, 1.0]))
    d3_is_one = abs(float(dk[2]) - 1.0) < 1e-12
    # CC_q shifts the a123 partial sum to the true codebook value; it is
    # injected through the lora ones-row so it never touches the compares.
    CCq = float(cb0) + (float(dk[1]) / 2.0 if d3_is_one else 0.0)

    xf = np.ascontiguousarray(x.reshape(M, I), dtype=np.float32)
    xt = (
        xf.reshape(NMSB, MSB, NG, GROUP)
        .transpose(0, 3, 2, 1)
        .astype(ml_dtypes.bfloat16)
        .reshape(NMSB * 128, NG, MSB)
    )
    scl_full = np.exp(np.ascontiguousarray(
        scale_log.reshape(O, NG), dtype=np.float64))  # [O, NG]
    lan = np.zeros((KAUG, I), dtype=np.float32)
    lan[:RANK] = np.ascontiguousarray(lora_A, dtype=np.float32)
    lan[RANK] = 1.0

    in_maps = []
    for c in range(N_CORES):
        sl = slice(c * OS, (c + 1) * OS)
        w_c = np.asarray(weight[sl], dtype=np.float64)          # [OS, I]
        scl_c = scl_full[sl]                                    # [OS, NG]
        # (W/S)^T per-group row blocks
        wns = (w_c.reshape(OS, NG, GROUP) / scl_c[:, :, None])  # [OS, NG, G]
        wnt = np.ascontiguousarray(
            wns.transpose(1, 2, 0).reshape(NG * GROUP, OS)).astype(np.float32)
        # recipS-scaled lora_B^T + CC row, per group
        lbt = (np.asarray(lora_B[sl], dtype=np.float64).T
               * ALPHA_OVER_RANK)                               # [RANK, OS]
        lbtn = np.empty((NG, KAUG, OS), dtype=np.float32)
        for g in range(NG):
            lbtn[g, :RANK] = (lbt / scl_c[:, g][None, :]).astype(np.float32)
            lbtn[g, RANK] = CCq
        in_maps.append({
            "xt": xt,
            "wnt": wnt,
            "sclt": np.ascontiguousarray(np.repeat(
                scl_c.T.astype(ml_dtypes.bfloat16)[:, None, :], 128, axis=1
            ).reshape(NG * 128, OS)),
            "lan": lan,
            "lbtn": lbtn,
        })
    return in_maps


def kernel(x, weight, scale_log, codebook, lora_A, lora_B):
    cb0, tk, dk = _codebook_consts(codebook)
    nc = _get_program(cb0, tk, dk)
    in_maps = _prep_in_maps(x, weight, scale_log, lora_A, lora_B, codebook)
    res = run_bass_kernel_spmd(nc, in_maps, core_ids=list(range(N_CORES))).results
    out = np.concatenate(
        [np.asarray(res[c]["out"]).astype(np.float32)
         for c in range(N_CORES)], axis=1)
    return out.reshape(x.shape[0], x.shape[1], O)
